# revision 2
# baseline (speedup 1.0000x reference)
"""Bilateral-filter L1 loss kernel v2 — Derivative_Erf pipeline.

Per unit: u = p - c (DVE); r' = derf(sqrt(g)*u) = 2/sqrt(pi)*exp(-g*u^2)
(ACT, one pass — replaces square+exp); m = r'*u (DVE, in-place).
W/U accumulate on PE with lhsT weights folded by sqrt(pi)/2.
Finalize: DVE recip + muls + reduce, GPSIMD dc/diff ops.
gps_m: number of units whose m-mul runs on GPSIMD instead of DVE.
"""

import numpy as np
from contextlib import ExitStack

B, C, H, W = 2, 3, 512, 512
KS, PAD = 5, 2
PW = W + 2 * PAD
NCORES = 8
RB = H // 128
NCH = B * C
PAIRS = NCH * RB
PPC = PAIRS // NCORES
UNITS = PPC * 2
FREE_IN = 5 * PW
ALPHA1, ALPHA2 = 0.1, 1.5
UROWS = 132 * PW
EW = 514
HALF = [(0, 1), (0, 2)] + [(di, dj) for di in (1, 2) for dj in (-2, -1, 0, 1, 2)]
NH = len(HALF)
SPI2 = float(np.sqrt(np.pi) / 2.0)

_cache = {}


def _spatial64():
    co = np.arange(-PAD, PAD + 1, dtype=np.float64)
    gy, gx = np.meshgrid(co, co, indexing="ij")
    return np.exp(-(gx ** 2 + gy ** 2) / (2.0 * ALPHA2)).reshape(-1)


def _slot(di, dj):
    if di == 0:
        return 0 if dj == 1 else 1
    return 2 + (di - 1) * 5 + (dj + 2)


def _sym_consts_v2():
    """idn2 [35,128,128], sidn [20,12,12], sel [12, UNITS*128] f16 lhsT
    constants, with derf's 2/sqrt(pi) compensated (x sqrt(pi)/2) on every
    r/m-consuming weight. idn2[0] (center, exact r=1) stays unscaled."""
    s64 = _spatial64().reshape(KS, KS)

    def sv(di, dj):
        return np.float16(s64[di + PAD, dj + PAD] * SPI2)

    idn2 = np.zeros((35, 128, 128), np.float16)
    np.fill_diagonal(idn2[0], np.float16(s64[PAD, PAD]))  # center: exact
    for t, (di, dj) in enumerate(HALF):
        np.fill_diagonal(idn2[1 + t], sv(di, dj))
        if di > 0:
            b = idn2[13 + (t - 2)]
            for m in range(di, 128):
                b[m - di, m] = sv(di, dj)
        n = idn2[23 + t]
        if di == 0:
            np.fill_diagonal(n, -sv(di, dj))
        else:
            for m in range(di, 128):
                n[m - di, m] = -sv(di, dj)
    sidn = np.zeros((20, 12, 12), np.float16)
    for t, (di, dj) in enumerate(HALF):
        if di == 0:
            continue
        st = t - 2
        for u in range(UNITS):
            for j in range(2):
                m = j - 2 + di
                if 0 <= m < 2:
                    sidn[st, 2 * u + j, 2 * u + m] = sv(di, dj)
                    sidn[10 + st, 2 * u + j, 2 * u + m] = -sv(di, dj)
    sel = np.zeros((12, UNITS * 128), np.float16)
    for u in range(UNITS):
        for j in range(2):
            sel[2 * u + j, u * 128 + j] = 1.0
    return idn2, sidn, sel


def _build_v5(a_out, a_tgt, repeat=1, gps_m=0, gps_fin=False, dma="sync",
              ablate=None):
    """v5: software-pipelined finalize (deferred one pair) so DVE never
    blocks on PE completion between pairs; unit-major matmuls with W/U
    PSUM tags bufs=3; dc computed early from xa (before tiles rotate)."""
    import concourse.bass as bass
    import concourse.bacc as bacc
    import concourse.tile as tile
    from concourse import mybir

    f16, f32 = mybir.dt.float16, mybir.dt.float32
    AF = mybir.ActivationFunctionType
    sg = (float(np.sqrt(a_out * a_out / (2.0 * ALPHA1))),
          float(np.sqrt(a_tgt * a_tgt / (2.0 * ALPHA1))))

    nc = bacc.Bacc("TRN2", target_bir_lowering=False, debug=False,
                   num_devices=NCORES)
    x = nc.dram_tensor("x", [UNITS * UROWS + 8], f16, kind="ExternalInput").ap()
    idn = nc.dram_tensor("idn", [35, 128, 128], f16, kind="ExternalInput").ap()
    sid = nc.dram_tensor("sid", [20, 12, 12], f16, kind="ExternalInput").ap()
    sel = nc.dram_tensor("sel", [12, UNITS * 128], f16,
                         kind="ExternalInput").ap()
    y = nc.dram_tensor("y", [128, PPC], f32, kind="ExternalOutput").ap()

    def win(t, off, dims):
        a = t[:]
        return bass.AP(a.tensor, a.offset + off, [list(a.ap[0])] + dims)

    def dram_ap(off, dims):
        a = x[:]
        return bass.AP(a.tensor, a.offset + off, dims)

    DJG = [(2, (0, 1, 2)), (1, (0, 1, 2)), (0, (1, 2)), (-1, (1, 2)),
           (-2, (1, 2))]

    with tile.TileContext(nc) as tc, ExitStack() as ctx:
        cpool = ctx.enter_context(tc.tile_pool(name="const", bufs=1))
        inp = ctx.enter_context(tc.tile_pool(name="inp", bufs=2))
        work = ctx.enter_context(tc.tile_pool(name="work", bufs=2))
        spool = ctx.enter_context(tc.tile_pool(name="spool", bufs=1))
        acc = ctx.enter_context(tc.tile_pool(name="acc", bufs=3, space="PSUM"))
        accf = ctx.enter_context(tc.tile_pool(name="accf", bufs=1,
                                              space="PSUM"))
        fin = ctx.enter_context(tc.tile_pool(name="fin", bufs=2))

        ident = cpool.tile([128, 35 * 128], f16)
        for k in range(35):
            nc.gpsimd.dma_start(ident[:, k * 128:(k + 1) * 128], idn[k])
        sids = cpool.tile([12, 20 * 12], f16)
        for k in range(20):
            nc.gpsimd.dma_start(sids[:, k * 12:(k + 1) * 12], sid[k])
        sels = cpool.tile([12, UNITS * 128], f16)
        nc.gpsimd.dma_start(sels[:], sel[:])
        ones = cpool.tile([128, W], f16)
        nc.vector.memset(ones[:], 1.0)
        loss_sb = cpool.tile([128, PPC], f32)

        def emit_u(dst, xa_t, xb_t, cen_t, cen_off, strip, eng):
            for dj, dis in DJG:
                dis = tuple(di for di in dis if (not strip or di > 0))
                cnt, di0 = len(dis), dis[0]
                if strip:
                    t0 = (di0 - 1) * 5 + (dj + 2)
                else:
                    t0 = _slot(di0, dj)
                coff = 0 if dj > 0 else 2
                poff = dj if dj > 0 else (dj + 2)
                if poff % 2 == 0:
                    src_t, pbase = xa_t, poff
                else:
                    src_t, pbase = xb_t, poff - 1
                rbase = di0 if strip else (di0 + 2)
                src = win(src_t, rbase * PW + pbase, [[PW, cnt], [1, EW]])
                cen = win(cen_t, cen_off + coff, [[0, cnt], [1, EW]])
                out = win(dst, t0 * EW, [[5 * EW, cnt], [1, EW]])
                eng.tensor_sub(out, src, cen)

        def body(_iv=None):
            deng = getattr(nc, dma)
            sin = spool.tile([12, 3 * PW], f16, tag="sin")
            sinb = spool.tile([12, 3 * PW], f16, tag="sinb")
            deng.dma_start(sin[:], dram_ap(
                0, [[UROWS, UNITS], [PW, 2], [1, 3 * PW]]))
            deng.dma_start(sinb[:], dram_ap(
                1, [[UROWS, UNITS], [PW, 2], [1, 3 * PW]]))
            su = spool.tile([12, 10 * EW], f16, tag="su")
            emit_u(su, sin, sinb, sin, 0, strip=True, eng=nc.vector)
            sr = spool.tile([12, 10 * EW], f16, tag="sr")
            nc.scalar.activation(sr[:], su[:], AF.Derivative_Erf,
                                 bias=0.0, scale=sg[0])
            nc.vector.tensor_mul(su[:], sr[:], su[:])

            fixW = accf.tile([12, W], f32, tag="fW")
            fixU = accf.tile([12, W], f32, tag="fU")
            stk = [t for t, (di, dj) in enumerate(HALF) if di > 0]
            for n, t in enumerate(stk):
                di, dj = HALF[t]
                st = t - 2
                o_s = st * EW + (2 - dj if dj > 0 else -dj)
                nc.tensor.matmul(fixW[:], sids[:, st * 12:(st + 1) * 12],
                                 sr[:, o_s:o_s + W],
                                 start=(n == 0), stop=(n == len(stk) - 1))
                nc.tensor.matmul(fixU[:], sids[:, (10 + st) * 12:(11 + st) * 12],
                                 su[:, o_s:o_s + W],
                                 start=(n == 0), stop=(n == len(stk) - 1))
            fxw = spool.tile([12, W], f16, tag="fxw")
            nc.vector.tensor_copy(fxw[:], fixW[:])
            fxu = spool.tile([12, W], f16, tag="fxu")
            nc.vector.tensor_copy(fxu[:], fixU[:])

            state = {}

            def emit_fin(p):
                Wo, Uo, Wt, Ut, dc = state.pop(p)
                feng = nc.gpsimd if gps_fin else nc.vector
                rw_o = fin.tile([128, W], f32, tag="rwo", name="rwo")
                nc.vector.reciprocal_approx_fast(rw_o[:], Wo[:])
                t_o = fin.tile([128, W], f32, tag="to", name="to")
                nc.vector.tensor_mul(t_o[:], Uo[:], rw_o[:])
                rw_t = fin.tile([128, W], f32, tag="rwt", name="rwt")
                nc.vector.reciprocal_approx_fast(rw_t[:], Wt[:])
                t_t = fin.tile([128, W], f32, tag="tt", name="tt")
                nc.vector.tensor_mul(t_t[:], Ut[:], rw_t[:])
                diff = fin.tile([128, W], f32, tag="diff", name="diff")
                feng.tensor_sub(diff[:], t_o[:], t_t[:])
                feng.tensor_add(diff[:], diff[:], dc[:])
                nc.vector.tensor_reduce(loss_sb[:, p:p + 1], diff[:],
                                        axis=mybir.AxisListType.X,
                                        op=mybir.AluOpType.add,
                                        apply_absolute_value=True)

            for pair in range(PPC):
                rs, ms, xas = [], [], []
                for img in range(2):
                    unit = pair * 2 + img
                    use_gps = ((gps_m == -1 and img == 0) or
                               (gps_m >= 1 and unit >= UNITS - gps_m))
                    xa = inp.tile([128, FREE_IN], f16, tag=f"xa{img}",
                                  name=f"xa{img}")
                    deng.dma_start(
                        xa[:], dram_ap(unit * UROWS, [[PW, 128], [1, FREE_IN]]))
                    xb = inp.tile([128, FREE_IN], f16, tag=f"xb{img}",
                                  name=f"xb{img}")
                    deng.dma_start(
                        xb[:], dram_ap(unit * UROWS + 1,
                                       [[PW, 128], [1, FREE_IN]]))
                    u = work.tile([128, NH * EW], f16, tag=f"u{img}",
                                  name=f"u{img}")
                    if ablate and 'dve' in ablate:
                        nc.vector.tensor_sub(u[:, 0:EW], xa[:, 0:EW],
                                             xa[:, 1:EW + 1])
                    else:
                        emit_u(u, xa, xb, xa, 2 * PW, strip=False,
                               eng=nc.vector)
                    r = work.tile([128, NH * EW], f16, tag=f"r{img}",
                                  name=f"r{img}")
                    if ablate and 'act' in ablate:
                        nc.scalar.activation(r[:, 0:EW], u[:, 0:EW],
                                             AF.Derivative_Erf,
                                             bias=0.0, scale=sg[img])
                    else:
                        nc.scalar.activation(r[:], u[:], AF.Derivative_Erf,
                                             bias=0.0, scale=sg[img])
                    meng = nc.gpsimd if use_gps else nc.vector
                    if ablate and 'dve' in ablate:
                        meng.tensor_mul(u[:, 0:EW], r[:, 0:EW], u[:, 0:EW])
                    else:
                        meng.tensor_mul(u[:], r[:], u[:])
                    rs.append(r)
                    ms.append(u)
                    xas.append(xa)
                    if img == 0 and pair >= 1:
                        emit_fin(pair - 1)

                dc = fin.tile([128, W], f32, tag="dc", name="dc")
                nc.vector.tensor_sub(dc, win(xas[0], 2 * PW + 2, [[1, W]]),
                                     win(xas[1], 2 * PW + 2, [[1, W]]))

                WUb = []
                for img in range(2):
                    Wp = acc.tile([128, W], f32, tag="W", name="Wp")
                    Up = acc.tile([128, W], f32, tag="U", name="Up")
                    nc.tensor.matmul(Wp[:], ident[:, 0:128], ones[:],
                                     start=True, stop=False)
                    taps = ([] if (ablate and 'mm' in ablate)
                            else list(enumerate(HALF)))
                    for t, (di, dj) in taps:
                        o_un = t * EW + (2 if dj > 0 else 0)
                        o_sh = t * EW + (2 - dj if dj > 0 else -dj)
                        lt_d = ident[:, (1 + t) * 128:(2 + t) * 128]
                        lt_sw = (lt_d if di == 0 else
                                 ident[:, (11 + t) * 128:(12 + t) * 128])
                        nc.tensor.matmul(Wp[:], lt_d,
                                         rs[img][:, o_un:o_un + W],
                                         start=False, stop=False)
                        nc.tensor.matmul(Wp[:], lt_sw,
                                         rs[img][:, o_sh:o_sh + W],
                                         start=False, stop=False)
                    for t, (di, dj) in taps:
                        o_un = t * EW + (2 if dj > 0 else 0)
                        o_sh = t * EW + (2 - dj if dj > 0 else -dj)
                        lt_d = ident[:, (1 + t) * 128:(2 + t) * 128]
                        lt_su = ident[:, (23 + t) * 128:(24 + t) * 128]
                        nc.tensor.matmul(Up[:], lt_d,
                                         ms[img][:, o_un:o_un + W],
                                         start=(t == 0), stop=False)
                        nc.tensor.matmul(Up[:], lt_su,
                                         ms[img][:, o_sh:o_sh + W],
                                         start=False, stop=False)
                    if ablate and 'mm' in ablate:
                        nc.tensor.matmul(Up[:], ident[:, 0:128], ones[:],
                                         start=True, stop=False)
                    unit = pair * 2 + img
                    usel = sels[:, unit * 128:(unit + 1) * 128]
                    nc.tensor.matmul(Wp[:], usel, fxw[:],
                                     start=False, stop=True)
                    nc.tensor.matmul(Up[:], usel, fxu[:],
                                     start=False, stop=True)
                    WUb.append((Wp, Up))

                state[pair] = (WUb[0][0], WUb[0][1], WUb[1][0], WUb[1][1], dc)
            emit_fin(PPC - 1)

        if repeat == 1:
            body()
        else:
            with tc.For_i(0, repeat, 1):
                body()
        nc.gpsimd.dma_start(y[:], loss_sb[:])

    nc.compile()
    return nc


def _build_v6(a_out, a_tgt, repeat=1, gps_m=0, gps_fin=False, dma="sync",
              ablate=None):
    """v6: group-level software pipelining. Each dj-group's sub feeds its
    derf and m immediately (5 groups x 2 images), so ACT starts ~4x
    earlier and the PE W/U loops trail group readiness. Finalize diffs in
    f16. Deferred finalize as in v5."""
    import concourse.bass as bass
    import concourse.bacc as bacc
    import concourse.tile as tile
    from concourse import mybir

    f16, f32 = mybir.dt.float16, mybir.dt.float32
    AF = mybir.ActivationFunctionType
    sg = (float(np.sqrt(a_out * a_out / (2.0 * ALPHA1))),
          float(np.sqrt(a_tgt * a_tgt / (2.0 * ALPHA1))))

    nc = bacc.Bacc("TRN2", target_bir_lowering=False, debug=False,
                   num_devices=NCORES)
    x = nc.dram_tensor("x", [UNITS * UROWS + 8], f16, kind="ExternalInput").ap()
    idn = nc.dram_tensor("idn", [35, 128, 128], f16, kind="ExternalInput").ap()
    sid = nc.dram_tensor("sid", [20, 12, 12], f16, kind="ExternalInput").ap()
    sel = nc.dram_tensor("sel", [12, UNITS * 128], f16,
                         kind="ExternalInput").ap()
    y = nc.dram_tensor("y", [128, PPC], f32, kind="ExternalOutput").ap()

    def win(t, off, dims):
        a = t[:]
        return bass.AP(a.tensor, a.offset + off, [list(a.ap[0])] + dims)

    def dram_ap(off, dims):
        a = x[:]
        return bass.AP(a.tensor, a.offset + off, dims)

    DJG = [(2, (0, 1, 2)), (1, (0, 1, 2)), (0, (1, 2)), (-1, (1, 2)),
           (-2, (1, 2))]
    # tap emission order: group-major (for PE to chase group readiness)
    GTAPS = []
    for dj, dis in DJG:
        for di in dis:
            t = _slot(di, dj)
            GTAPS.append((t, (di, dj)))

    with tile.TileContext(nc) as tc, ExitStack() as ctx:
        cpool = ctx.enter_context(tc.tile_pool(name="const", bufs=1))
        inp = ctx.enter_context(tc.tile_pool(name="inp", bufs=2))
        work = ctx.enter_context(tc.tile_pool(name="work", bufs=2))
        spool = ctx.enter_context(tc.tile_pool(name="spool", bufs=1))
        acc = ctx.enter_context(tc.tile_pool(name="acc", bufs=3, space="PSUM"))
        accf = ctx.enter_context(tc.tile_pool(name="accf", bufs=1,
                                              space="PSUM"))
        fin = ctx.enter_context(tc.tile_pool(name="fin", bufs=2))

        ident = cpool.tile([128, 35 * 128], f16)
        for k in range(35):
            nc.gpsimd.dma_start(ident[:, k * 128:(k + 1) * 128], idn[k])
        sids = cpool.tile([12, 20 * 12], f16)
        for k in range(20):
            nc.gpsimd.dma_start(sids[:, k * 12:(k + 1) * 12], sid[k])
        sels = cpool.tile([12, UNITS * 128], f16)
        nc.gpsimd.dma_start(sels[:], sel[:])
        ones = cpool.tile([128, W], f16)
        nc.vector.memset(ones[:], 1.0)
        loss_sb = cpool.tile([128, PPC], f32)

        def group_aps(g, u_t, r_t, xa_t, xb_t, strip=False):
            dj, dis = DJG[g]
            dis = tuple(di for di in dis if (not strip or di > 0))
            cnt = len(dis)
            di0 = dis[0]
            if strip:
                t0 = (di0 - 1) * 5 + (dj + 2)
            else:
                t0 = _slot(di0, dj)
            coff = 0 if dj > 0 else 2
            poff = dj if dj > 0 else (dj + 2)
            if poff % 2 == 0:
                src_t, pbase = xa_t, poff
            else:
                src_t, pbase = xb_t, poff - 1
            rbase = di0 if strip else (di0 + 2)
            src = win(src_t, rbase * PW + pbase, [[PW, cnt], [1, EW]])
            cen = win(xa_t,
                      (2 * PW if not strip else 0) + coff, [[0, cnt], [1, EW]])
            uap = win(u_t, t0 * EW, [[5 * EW, cnt], [1, EW]])
            rap = win(r_t, t0 * EW, [[5 * EW, cnt], [1, EW]]) if r_t else None
            return src, cen, uap, rap

        def body(_iv=None):
            deng = getattr(nc, dma)
            sin = spool.tile([12, 3 * PW], f16, tag="sin")
            sinb = spool.tile([12, 3 * PW], f16, tag="sinb")
            deng.dma_start(sin[:], dram_ap(
                0, [[UROWS, UNITS], [PW, 2], [1, 3 * PW]]))
            deng.dma_start(sinb[:], dram_ap(
                1, [[UROWS, UNITS], [PW, 2], [1, 3 * PW]]))
            su = spool.tile([12, 10 * EW], f16, tag="su")
            for g in range(5):
                src, cen, uap, _ = group_aps(g, su, None, sin, sinb,
                                             strip=True)
                nc.vector.tensor_sub(uap, src, cen)
            sr = spool.tile([12, 10 * EW], f16, tag="sr")
            nc.scalar.activation(sr[:], su[:], AF.Derivative_Erf,
                                 bias=0.0, scale=sg[0])
            nc.vector.tensor_mul(su[:], sr[:], su[:])

            fixW = accf.tile([12, W], f32, tag="fW")
            fixU = accf.tile([12, W], f32, tag="fU")
            stk = [t for t, (di, dj) in enumerate(HALF) if di > 0]
            for n, t in enumerate(stk):
                di, dj = HALF[t]
                st = t - 2
                o_s = st * EW + (2 - dj if dj > 0 else -dj)
                nc.tensor.matmul(fixW[:], sids[:, st * 12:(st + 1) * 12],
                                 sr[:, o_s:o_s + W],
                                 start=(n == 0), stop=(n == len(stk) - 1))
                nc.tensor.matmul(fixU[:], sids[:, (10 + st) * 12:(11 + st) * 12],
                                 su[:, o_s:o_s + W],
                                 start=(n == 0), stop=(n == len(stk) - 1))
            fxw = spool.tile([12, W], f16, tag="fxw")
            nc.vector.tensor_copy(fxw[:], fixW[:])
            fxu = spool.tile([12, W], f16, tag="fxu")
            nc.vector.tensor_copy(fxu[:], fixU[:])

            state = {}

            def emit_fin(p):
                Wo, Uo, Wt, Ut, dc = state.pop(p)
                rw_o = fin.tile([128, W], f32, tag="rwo", name="rwo")
                nc.vector.reciprocal_approx_fast(rw_o[:], Wo[:])
                t_o = fin.tile([128, W], f16, tag="to", name="to")
                nc.vector.tensor_mul(t_o[:], Uo[:], rw_o[:])
                rw_t = fin.tile([128, W], f32, tag="rwt", name="rwt")
                nc.vector.reciprocal_approx_fast(rw_t[:], Wt[:])
                t_t = fin.tile([128, W], f16, tag="tt", name="tt")
                nc.vector.tensor_mul(t_t[:], Ut[:], rw_t[:])
                diff = fin.tile([128, W], f16, tag="diff", name="diff")
                nc.vector.tensor_sub(diff[:], t_o[:], t_t[:])
                nc.vector.tensor_add(diff[:], diff[:], dc[:])
                nc.vector.tensor_reduce(loss_sb[:, p:p + 1], diff[:],
                                        axis=mybir.AxisListType.X,
                                        op=mybir.AluOpType.add,
                                        apply_absolute_value=True)

            for pair in range(PPC):
                rs, ms, xas, xbs = [], [], [], []
                for img in range(2):
                    unit = pair * 2 + img
                    xa = inp.tile([128, FREE_IN], f16, tag=f"xa{img}",
                                  name=f"xa{img}")
                    deng.dma_start(
                        xa[:], dram_ap(unit * UROWS, [[PW, 128], [1, FREE_IN]]))
                    xb = inp.tile([128, FREE_IN], f16, tag=f"xb{img}",
                                  name=f"xb{img}")
                    deng.dma_start(
                        xb[:], dram_ap(unit * UROWS + 1,
                                       [[PW, 128], [1, FREE_IN]]))
                    u = work.tile([128, NH * EW], f16, tag=f"u{img}",
                                  name=f"u{img}")
                    r = work.tile([128, NH * EW], f16, tag=f"r{img}",
                                  name=f"r{img}")
                    rs.append(r)
                    ms.append(u)
                    xas.append(xa)
                    xbs.append(xb)
                # group-pipelined elementwise, images interleaved
                for g in range(5):
                    for img in range(2):
                        src, cen, uap, rap = group_aps(
                            g, ms[img], rs[img], xas[img], xbs[img])
                        nc.vector.tensor_sub(uap, src, cen)
                    for img in range(2):
                        _, _, uap, rap = group_aps(g, ms[img], rs[img],
                                                   xas[img], xbs[img])
                        nc.scalar.activation(rap, uap, AF.Derivative_Erf,
                                             bias=0.0, scale=sg[img])
                    for img in range(2):
                        _, _, uap, rap = group_aps(g, ms[img], rs[img],
                                                   xas[img], xbs[img])
                        nc.vector.tensor_mul(uap, rap, uap)
                if pair >= 1:
                    emit_fin(pair - 1)
                dc = fin.tile([128, W], f16, tag="dc", name="dc")
                nc.vector.tensor_sub(dc, win(xas[0], 2 * PW + 2, [[1, W]]),
                                     win(xas[1], 2 * PW + 2, [[1, W]]))

                WUb = []
                for img in range(2):
                    Wp = acc.tile([128, W], f32, tag="W", name="Wp")
                    Up = acc.tile([128, W], f32, tag="U", name="Up")
                    nc.tensor.matmul(Wp[:], ident[:, 0:128], ones[:],
                                     start=True, stop=False)
                    for t, (di, dj) in GTAPS:
                        o_un = t * EW + (2 if dj > 0 else 0)
                        o_sh = t * EW + (2 - dj if dj > 0 else -dj)
                        lt_d = ident[:, (1 + t) * 128:(2 + t) * 128]
                        lt_sw = (lt_d if di == 0 else
                                 ident[:, (11 + t) * 128:(12 + t) * 128])
                        nc.tensor.matmul(Wp[:], lt_d,
                                         rs[img][:, o_un:o_un + W],
                                         start=False, stop=False)
                        nc.tensor.matmul(Wp[:], lt_sw,
                                         rs[img][:, o_sh:o_sh + W],
                                         start=False, stop=False)
                    for n, (t, (di, dj)) in enumerate(GTAPS):
                        o_un = t * EW + (2 if dj > 0 else 0)
                        o_sh = t * EW + (2 - dj if dj > 0 else -dj)
                        lt_d = ident[:, (1 + t) * 128:(2 + t) * 128]
                        lt_su = ident[:, (23 + t) * 128:(24 + t) * 128]
                        nc.tensor.matmul(Up[:], lt_d,
                                         ms[img][:, o_un:o_un + W],
                                         start=(n == 0), stop=False)
                        nc.tensor.matmul(Up[:], lt_su,
                                         ms[img][:, o_sh:o_sh + W],
                                         start=False, stop=False)
                    unit = pair * 2 + img
                    usel = sels[:, unit * 128:(unit + 1) * 128]
                    nc.tensor.matmul(Wp[:], usel, fxw[:],
                                     start=False, stop=True)
                    nc.tensor.matmul(Up[:], usel, fxu[:],
                                     start=False, stop=True)
                    WUb.append((Wp, Up))

                state[pair] = (WUb[0][0], WUb[0][1], WUb[1][0], WUb[1][1], dc)
            emit_fin(PPC - 1)

        if repeat == 1:
            body()
        else:
            with tc.For_i(0, repeat, 1):
                body()
        nc.gpsimd.dma_start(y[:], loss_sb[:])

    nc.compile()
    return nc


def _host_shards(output, target):
    s = _spatial64()
    xs = []
    for arr in (output, target):
        pad = np.pad(arr.reshape(NCH, H, W),
                     ((0, 0), (PAD, PAD), (PAD, PAD)), mode="reflect")
        xs.append(pad.astype(np.float16))

    idn2, sidn, sel = _sym_consts_v2()
    in_maps = []
    for core in range(NCORES):
        xc = np.zeros(UNITS * UROWS + 8, np.float16)
        xv = xc[:UNITS * UROWS].reshape(UNITS, UROWS)
        for p in range(PPC):
            bc, rb = divmod(core * PPC + p, RB)
            for img in (0, 1):
                blk = xs[img][bc][rb * 128: rb * 128 + 132]
                xv[p * 2 + img] = blk.reshape(-1)
        in_maps.append({"x": xc, "idn": idn2, "sid": sidn, "sel": sel})
    return in_maps


def _numpy_fallback(output, target):
    def filt(img):
        a = 0.5 if img.min() < 0 else 1.0
        img01 = a * img + (0.5 if a == 0.5 else 0.0)
        pad = np.pad(img01, ((0, 0), (0, 0), (PAD, PAD), (PAD, PAD)),
                     mode="reflect")
        pat = np.stack([pad[:, :, i:i + H, j:j + W]
                        for i in range(KS) for j in range(KS)], 2)
        cen = img01[:, :, None]
        s = _spatial64()[None, None, :, None, None]
        w = np.exp(-(pat - cen) ** 2 / (2 * ALPHA1)) * s
        return (w * pat).sum(2) / (w.sum(2) + 1e-8)

    o = filt(output.astype(np.float64))
    t = filt(target.astype(np.float64))
    return np.float32(np.abs(o - t).mean())


BUILDER = "v6"


def _build_best(a_o, a_t, repeat=1):
    b = _build_v6 if BUILDER == "v6" else _build_v5
    return b(a_o, a_t, repeat=repeat)


def kernel(output, target):
    from concourse.bass_utils import run_bass_kernel_spmd

    output = np.asarray(output, np.float32)
    target = np.asarray(target, np.float32)
    a_o = 0.5 if output.min() < 0 else 1.0
    a_t = 0.5 if target.min() < 0 else 1.0
    if a_o != a_t:
        return _numpy_fallback(output, target)

    key = (a_o, a_t, BUILDER)
    if key not in _cache:
        _cache[key] = _build_best(a_o, a_t)
    nc = _cache[key]

    in_maps = _host_shards(output, target)
    res = run_bass_kernel_spmd(nc, in_maps, list(range(NCORES)))
    total = np.float64(0.0)
    for r in res.results:
        total += r["y"].astype(np.float64).sum()
    loss = a_o * total / (B * C * H * W)
    return np.float32(loss)
